# revision 53
# baseline (speedup 1.0000x reference)
"""Multi-head attention (B=4, Q=K=2048, N=12 heads, H=64) on 8 TRN2 NeuronCores.

Sharding: core c handles batch b = c // 2 and head-group g = c % 2 (6 local
heads, output columns [g*384:(g+1)*384]). Pure data-parallel, no collectives.

v9 design (evolved from the v3 baseline via NTFF trace analysis; measures
~255us with the same tracing methodology that measured v3 at 328us):
  - Zero-padded per-(head, sub) q/k tiles: every matmul (scores, PV,
    projections) is a plain 128-deep full-array op, so the PE never
    changes tiling mode.  The 64x128 row-tiled score pairs of v3 never
    actually overlapped in production (trace-verified) and the constant
    (64,128)<->(128,128) mode flips cost ~100ns of drain per switch.
  - Split-sub score emission: each head's score matmuls are WAR-gated only
    on that head's own previous exp read (one sub-step earlier), so they
    run inside the other head's exp window and the Act engine (exp) never
    stalls on PE.
  - Wide-ones PV: v tiles are [128 = 64 v-dims | 64 ones] per head, so
    PSUM rows 64-127 of the PV accumulator hold broadcast copies of the
    softmax denominator.  The finish is: fast DVE copies (release the PSUM
    bank), one partition-stacked [128,512] reciprocal per head-pair in
    [128,128] quarters (DVE reciprocal costs ~6.3 cyc per FREE element,
    so partition-stacking both heads halves it), a DVE multiply, and a DMA
    to a TRANSPOSED DRAM output [384, 2048] (host assembles with .T).
    No PE transposes at all.  Finish work is drip-fed (<=2 DVE ops per
    stream entry) so it never delays the projection casts that gate PE.
  - 8 chunks of 2 k-tiles ([128,1024] score tiles = 2 PSUM banks) leave
    4 banks for DOUBLE-buffered projection and PV accumulators -- worth
    ~30us: with single buffers the Tile scheduler serializes the next
    (head, qb) PV/projection group behind the previous group's PSUM reads.
  - Inputs arrive host-packed chunk-major bf16 (6KB contiguous per
    partition per DMA) with weights dt-major; the prologue projects only
    what the first score chunk needs, and the remaining q/k projection
    fillers are spread as late as their score deadlines allow -- packing
    them early crowded the first third of the stream and stalled Act.
"""

import sys
from contextlib import ExitStack

sys.path.insert(0, "/opt/trn_rl_repo")

import numpy as np
import ml_dtypes

import concourse.bass as bass
import concourse.tile as tile
from concourse import bacc, mybir
from concourse.bass_utils import run_bass_kernel_spmd

F32 = mybir.dt.float32
BF16 = mybir.dt.bfloat16
EXPF = mybir.ActivationFunctionType.Exp
MUL = mybir.AluOpType.mult
DIV = mybir.AluOpType.divide

B, SEQ, N_HEADS, H = 4, 2048, 12, 64
D = N_HEADS * H            # 768
NH = 6                     # heads per core
NM = NH // 2               # head pairs (m-tiles)
DG = NH * H                # 384 output cols per core
P = 128
DT = D // P                # 6 d-tiles
QB = SEQ // 512            # 4 q blocks of 512
CHUNKS = (2, 2, 2, 2, 2, 2, 2, 2)   # k-tiles per chunk (exp width 1024)
NCH = len(CHUNKS)
RT = SEQ // P              # 16 k row tiles
E_LAG = 10                 # PV trails exp by this many stream entries
E_LAG_LATE = 3
LAG_SWITCH = 64
SCALE = 0.125              # 1/sqrt(64)
USE_DIVIDE = False         # DVE divide rejected by BIR verifier; recip+mul

NPBF16 = ml_dtypes.bfloat16


def build_nc(reps: int = 1, diag: str = ""):
    nc = bacc.Bacc("TRN2", target_bir_lowering=False, debug=False, num_devices=8)

    # chunk-major packed inputs: [ch][p][dt*512] -> 6KB contiguous per
    # partition per chunk, one fat DMA per (tensor, chunk)
    xq_d = nc.dram_tensor("xqT", [QB, P, DT * 512], BF16,
                          kind="ExternalInput").ap()
    xk_d = nc.dram_tensor("xkT", [QB, P, DT * 512], BF16,
                          kind="ExternalInput").ap()
    xv_d = nc.dram_tensor("xvT", [QB, P, DT * 512], BF16,
                          kind="ExternalInput").ap()
    x_d = {"q": xq_d, "k": xk_d, "v": xv_d}
    wq_d = nc.dram_tensor("wq", [P, DT * DG], BF16, kind="ExternalInput").ap()
    wk_d = nc.dram_tensor("wk", [P, DT * DG], BF16, kind="ExternalInput").ap()
    wv_d = nc.dram_tensor("wv", [P, DT * DG], BF16, kind="ExternalInput").ap()
    out_d = nc.dram_tensor("out", [DG, SEQ], F32, kind="ExternalOutput").ap()

    with tile.TileContext(nc) as tc:
     for _rep in range(reps):
      with ExitStack() as stack:
        singles = stack.enter_context(tc.tile_pool(name="singles", bufs=1))
        w_sb = {}
        for t in ("q", "k", "v"):
            w_sb[t] = singles.tile([P, DT, DG], BF16, tag=f"w{t}", name=f"w{t}")

        xTp = stack.enter_context(tc.tile_pool(name="xT", bufs=1))
        # per (tensor, chunk): [128, dt, 512] bf16
        xch = {(t, ch): xTp.tile([P, DT, 512], BF16, tag=f"{t}C{ch}",
                                 name=f"{t}C{ch}")
               for t in ("k", "q", "v") for ch in range(QB)}

        # projected q/k per (m-tile, sub-head): [128, seq] bf16 with the
        # OTHER head's 64 partitions zeroed.  Score matmuls are then plain
        # 128-deep full-array ops (zero rows contribute nothing), so the PE
        # never changes tiling mode -- no drain penalties, no tile_position.
        qkp = {(t, m, s): singles.tile([P, SEQ], BF16, tag=f"{t}m{m}s{s}",
                                       name=f"{t}m{m}s{s}")
               for t in ("q", "k") for m in range(NM) for s in (0, 1)}

        # v with wide-ones: [:, h, 0:64] = projected v, [:, h, 64:128] = 1.0
        vpool = stack.enter_context(tc.tile_pool(name="v", bufs=1))
        v_sb = [vpool.tile([P, NH, P], BF16, tag=f"v{kt}", name=f"v{kt}")
                for kt in range(RT)]
        for kt in range(RT):
            nc.gpsimd.memset(v_sb[kt][:, :, H:P], 1.0)

        # ---- input loads: k/w on the SP queue, q/v on the gpsimd (SWDGE)
        # queue so the two streams transfer in parallel ------------------
        def x_load_chunk(t, ch, eng):
            eng.dma_start(
                out=xch[t, ch].rearrange("p dt c -> p (dt c)"),
                in_=x_d[t][ch])

        def w_load(t, wd, m, eng):
            # m-tile slice: cols [m*128, (m+1)*128) of each dt block
            eng.dma_start(
                out=w_sb[t][:, :, m * P:(m + 1) * P],
                in_=wd.rearrange("p (dt c) -> p dt c", dt=DT)[:, :,
                                                             m * P:(m + 1) * P])
        w_load("k", wk_d, 0, nc.sync)
        x_load_chunk("k", 0, nc.sync)
        w_load("q", wq_d, 0, nc.sync)
        x_load_chunk("q", 0, nc.sync)
        for ch in range(1, 4):
            x_load_chunk("k", ch, nc.sync)
            x_load_chunk("q", ch, nc.sync)
        nc.sync.dma_start(out=w_sb["v"].rearrange("p dt c -> p (dt c)"),
                          in_=wv_d)
        for ch in range(4):
            x_load_chunk("v", ch, nc.sync)
        for m in range(1, NM):
            w_load("k", wk_d, m, nc.sync)
            w_load("q", wq_d, m, nc.sync)
        # zero the unused half of each per-sub q/k tile (after the DMA
        # triggers so the transfers start immediately; m0 first -- the
        # first scores need those)
        for m in range(NM):
            for t in ("k", "q"):
                nc.gpsimd.memset(qkp[t, m, 0][64:P, :], 0.0)
                nc.gpsimd.memset(qkp[t, m, 1][0:64, :], 0.0)

        # ---- main pools ----------------------------------------------------
        psProj = stack.enter_context(tc.tile_pool(name="psProj", bufs=1,
                                                  space="PSUM"))
        psS = stack.enter_context(tc.tile_pool(name="psS", bufs=1, space="PSUM"))
        psPV = stack.enter_context(tc.tile_pool(name="psPV", bufs=1,
                                                space="PSUM"))
        expp = stack.enter_context(tc.tile_pool(name="expp", bufs=E_LAG + 4))
        outp = stack.enter_context(tc.tile_pool(name="outp", bufs=3))
        smallp = stack.enter_context(tc.tile_pool(name="small", bufs=4))

        # ---- emission helpers ---------------------------------------------
        def proj_m(t, m, ch):
            """Project q/k m-tile chunk: q/k-range [ch*512,(ch+1)*512)."""
            pj = psProj.tile([P, 512], F32, tag="pj", name=f"pj{t}{m}{ch}", bufs=2)
            for dt in range(DT):
                nc.tensor.matmul(
                    pj, w_sb[t][:, dt, m * P:(m + 1) * P],
                    xch[t, ch][:, dt, :],
                    start=(dt == 0), stop=(dt == DT - 1))
            cs = slice(ch * 512, (ch + 1) * 512)
            nc.vector.tensor_copy(out=qkp[t, m, 0][0:64, cs], in_=pj[0:64, :])
            nc.vector.tensor_copy(out=qkp[t, m, 1][64:P, cs], in_=pj[64:P, :])

        def vproj_chunk(kt):
            pj = psProj.tile([P, 512], F32, tag="pj", name=f"pjv{kt}", bufs=2)
            ch, kk = divmod(kt, 4)
            for dt in range(DT):
                nc.tensor.matmul(pj[:, 0:DG],
                                 xch["v", ch][:, dt, kk * P:(kk + 1) * P],
                                 w_sb["v"][:, dt, :],
                                 start=(dt == 0), stop=(dt == DT - 1))
            nc.vector.tensor_copy(
                out=v_sb[kt][:, :, 0:H],
                in_=pj[:, 0:DG].rearrange("p (n h) -> p n h", h=H))

        # finish work is deferred and drip-fed (<=2 DVE ops per stream
        # entry) so its ~5us DVE burst never delays the projection casts
        # that gate PE work -- a burst at each (m,qb) boundary was stalling
        # the PE ~2.5us and re-throttling the HAM clock.
        dve_q = []

        def finish_pair(m, qb, pvA, pvB):
            """pv [128,512]: rows 0-63 context, 64-127 denominator copies.

            Fast DVE copies release the PSUM pv slots (keeps PE fed and the
            HAM clock warm).  The reciprocal costs ~6.3 DVE cycles per FREE
            element regardless of partitions, so both heads' denominators
            are stacked into one [128,512] tile and the reciprocal runs in
            [128,128] quarters to keep DVE queue blockages short.
            """
            hA, hB = 2 * m, 2 * m + 1
            num = outp.tile([P, 512], F32, tag="num", name=f"n{m}{qb}",
                            bufs=3)
            den = smallp.tile([P, 512], F32, tag="den", name=f"d{m}{qb}",
                              bufs=2)
            for pv, lo in ((pvA, 0), (pvB, H)):
                nc.vector.tensor_copy(out=num[lo:lo + H, :], in_=pv[0:H, :])
                nc.vector.tensor_copy(out=den[lo:lo + H, :], in_=pv[H:P, :])
            rcp = smallp.tile([P, 512], F32, tag="rcp", name="rcp", bufs=2)

            def rq(qq):
                s = slice(qq * P, (qq + 1) * P)
                return lambda: nc.vector.reciprocal(rcp[:, s], den[:, s])

            def mq(h, lo, osb):
                def go():
                    nc.vector.tensor_tensor(out=osb[lo:lo + H, :],
                                            in0=num[lo:lo + H, :],
                                            in1=rcp[lo:lo + H, :], op=MUL)
                    nc.sync.dma_start(
                        out=out_d[h * H:(h + 1) * H,
                                  qb * 512:(qb + 1) * 512],
                        in_=osb[lo:lo + H, :])
                return go

            osb = outp.tile([P, 512], F32, tag="osb", name=f"o{m}{qb}",
                            bufs=3)
            ops = [rq(qq) for qq in range(4)] + [mq(hA, 0, osb),
                                                mq(hB, H, osb)]
            if qb == QB - 1 and m >= NM - 2:
                for op in ops:     # last pairs: straight-line, no deferral
                    op()
            else:
                dve_q.extend(ops)

        # ---- filler schedule (slot = stream entry index) -------------------
        # stream: (m, qb, c, sub) -> 12 entries per (m, qb); m0 spans
        # entries 0-47, m1 48-95, m2 96-143.
        def pj_item(t, m, ch):
            return lambda: proj_m(t, m, ch)

        def vp(k):
            return lambda: vproj_chunk(k)

        sched = {2: [pj_item("q", 0, 1)], 13: [pj_item("q", 0, 2)],
                 25: [pj_item("q", 0, 3)]}
        for k in range(RT):
            # vp k must land by stream entry 2*(k//3) + E_LAG (PV deadline;
            # sched items run before drain_pv within an entry)
            slot = 4 + k
            assert slot <= 2 * (k // 2) + E_LAG
            sched.setdefault(slot, []).append(vp(k))
        later = ([("k", 1, c) for c in range(4)] +
                 [("q", 1, c) for c in range(4)] +
                 [("k", 2, c) for c in range(4)] +
                 [("q", 2, c) for c in range(4)])
        slots_later = (26, 34, 42, 50, 56, 66, 76, 86,
                       94, 102, 110, 118, 126, 134, 142, 150)
        for i, (t, m, c) in enumerate(later):
            # deadlines: k-m1 by 64, q-m1 by 64+16c, k-m2 by 128, q-m2 by
            # 128+16c; each slot leaves >=8 entries of margin
            sched.setdefault(slots_later[i], []).append(pj_item(t, m, c))

        # ---- prologue: only what chunk-0 scores need; rest via sched ------
        proj_m("k", 0, 0)
        proj_m("q", 0, 0)
        # k m0 chunk ch feeds score chunks with k-tiles in [4ch, 4ch+4):
        # score chunk c uses kt 3c..3c+2 -> k-chunk 1 by entry 2, 2 by 4,
        # 3 by 8 (q m0 later chunks already in sched at 2/13/25)
        sched.setdefault(1, []).insert(0, pj_item("k", 0, 1))
        sched.setdefault(3, []).insert(0, pj_item("k", 0, 2))
        sched.setdefault(6, []).insert(0, pj_item("k", 0, 3))

        # ---- main loop -----------------------------------------------------
        stream = [(m, qb, c, sub) for m in range(NM) for qb in range(QB)
                  for c in range(NCH) for sub in (0, 1)]
        pv_tiles = {}
        e_tiles = {}

        KT0 = [sum(CHUNKS[:i]) for i in range(NCH)]

        def emit_pv(m, qb, c, sub):
            h = 2 * m + sub
            if (h, qb) not in pv_tiles:
                pv_tiles[h, qb] = psPV.tile([P, 512], F32, tag="pv",
                                            name=f"pv{h}{qb}", bufs=2)
            pv = pv_tiles[h, qb]
            e = e_tiles.pop((h, qb, c))
            for j in range(CHUNKS[c]):
                kt = KT0[c] + j
                nc.tensor.matmul(pv, v_sb[kt][:, h, :],
                                 e[:, j * 512:(j + 1) * 512],
                                 start=(kt == 0), stop=(kt == RT - 1))
            if c == NCH - 1 and sub == 1:
                finish_pair(m, qb, pv_tiles.pop((2 * m, qb)),
                            pv_tiles.pop((2 * m + 1, qb)))

        pv_next = [0]

        def drain_pv(idx):
            lag = E_LAG if idx < LAG_SWITCH else E_LAG_LATE
            limit = 3
            n = 0
            while pv_next[0] <= idx - lag and n < limit:
                emit_pv(*stream[pv_next[0]])
                pv_next[0] += 1
                n += 1

        # Each sub-head's score matmuls for chunk c are emitted at its own
        # sub step, WAR-gated only on THIS head's previous exp read (which
        # completed one sub-step earlier), so they run inside the other
        # head's exp window and the Act engine never stalls.  Zero-padded
        # qkp tiles make these plain 128-deep matmuls: no mode switches.
        for idx, (m, qb, c, sub) in enumerate(stream):
            csz = CHUNKS[c]
            h = 2 * m + sub
            sval = psS.tile([P, 1024], F32, tag=("sA" if sub == 0 else "sB"),
                            name=f"s{h}{qb}{c}")
            kTm, qTm = qkp["k", m, sub], qkp["q", m, sub]
            for j in range(csz):
                kt = KT0[c] + j
                nc.tensor.matmul(
                    sval[:, j * 512:(j + 1) * 512],
                    kTm[:, kt * P:(kt + 1) * P],
                    qTm[:, qb * 512:(qb + 1) * 512],
                    start=True, stop=True)
            e = expp.tile([P, 1024], BF16, tag="e", name="e")
            nc.scalar.activation(out=e[:, 0:csz * 512],
                                 in_=sval[:, 0:csz * 512],
                                 func=EXPF, scale=SCALE)
            e_tiles[h, qb, c] = e

            for item in sched.get(idx, ()):
                item()
            drain_pv(idx)
            for _ in range(6 if idx >= 168 else 2):
                if dve_q:
                    dve_q.pop(0)()

        while pv_next[0] < len(stream):
            emit_pv(*stream[pv_next[0]])
            pv_next[0] += 1
        while dve_q:
            dve_q.pop(0)()

    nc.compile()
    return nc


_NC_CACHE = {}


def _get_nc(has_bias=False, has_mask=False, reps=1, diag=""):
    assert not has_bias and not has_mask
    key = (reps, diag)
    if key not in _NC_CACHE:
        _NC_CACHE[key] = build_nc(reps, diag)
    return _NC_CACHE[key]


def _host_dtmajor(W):
    """[768, C] -> partition-major [128, 6*C]: row p holds dt-tile rows."""
    C = W.shape[1]
    return np.ascontiguousarray(
        W.reshape(DT, P, C).transpose(1, 0, 2).reshape(P, DT * C))


def _host_chunkmajor(xb16_rows):
    """x [SEQ, D] bf16 -> [QB, P, DT*512]: chunk-major, 6KB/partition DMAs."""
    xT = xb16_rows.T  # [768, 2048]
    return np.ascontiguousarray(
        xT.reshape(DT, P, QB, 512).transpose(2, 1, 0, 3).reshape(
            QB, P, DT * 512))


def shard_inputs(query, key, value, mask, Wq, bq, Wk, bk, Wv, bv,
                 batch_size=B, num_heads=N_HEADS):
    query = np.asarray(query, dtype=np.float32)
    key = np.asarray(key, dtype=np.float32)
    value = np.asarray(value, dtype=np.float32)
    Wq = np.asarray(Wq, dtype=np.float32)
    Wk = np.asarray(Wk, dtype=np.float32)
    Wv = np.asarray(Wv, dtype=np.float32)
    assert query.shape == (B * SEQ, D) and key.shape == (B * SEQ, D)
    assert int(batch_size) == B and int(num_heads) == N_HEADS

    has_bias = bool(np.any(bq) or np.any(bk) or np.any(bv))
    has_mask = bool(np.any(mask))

    qb16 = query.astype(NPBF16)
    kb16 = key.astype(NPBF16)
    vb16 = value.astype(NPBF16)

    in_maps = []
    for c in range(8):
        b, g = divmod(c, 2)
        rows = slice(b * SEQ, (b + 1) * SEQ)
        cols = slice(g * DG, (g + 1) * DG)
        m = {
            "xqT": _host_chunkmajor(qb16[rows]),
            "xkT": _host_chunkmajor(kb16[rows]),
            "xvT": _host_chunkmajor(vb16[rows]),
            "wq": _host_dtmajor(Wq[:, cols]).astype(NPBF16),
            "wk": _host_dtmajor(Wk[:, cols]).astype(NPBF16),
            "wv": _host_dtmajor(Wv[:, cols]).astype(NPBF16),
        }
        in_maps.append(m)
    return in_maps, has_bias, has_mask


def make_in_maps(inputs):
    return shard_inputs(**{k: inputs[k] for k in
                           ("query", "key", "value", "mask", "Wq", "bq",
                            "Wk", "bk", "Wv", "bv", "batch_size", "num_heads")})[0]


def assemble(results):
    full = np.empty((B * SEQ, D), dtype=np.float32)
    for c in range(8):
        b, g = divmod(c, 2)
        full[b * SEQ:(b + 1) * SEQ, g * DG:(g + 1) * DG] = results[c]["out"].T
    return full


def _reference_fallback(query, key, value, mask, Wq, bq, Wk, bk, Wv, bv,
                        batch_size, num_heads):
    b, n = int(batch_size), int(num_heads)
    d = Wq.shape[1]
    h = d // n
    q_len = query.shape[0] // b
    k_len = key.shape[0] // b
    q = (query @ Wq + bq).reshape(b, q_len, n, h).transpose(0, 2, 1, 3)
    k = (key @ Wk + bk).reshape(b, k_len, n, h).transpose(0, 2, 1, 3)
    v = (value @ Wv + bv).reshape(b, k_len, n, h).transpose(0, 2, 1, 3)
    s = np.einsum('bnqh,bnkh->bnqk', q, k) / np.sqrt(h).astype(np.float32)
    s = s + mask
    s = s - s.max(-1, keepdims=True)
    w = np.exp(s)
    w /= w.sum(-1, keepdims=True)
    c = np.einsum('bnqk,bnkh->bqnh', w, v)
    return c.reshape(b * q_len, n * h).astype(np.float32)


def kernel(query, key, value, mask, Wq, bq, Wk, bk, Wv, bv,
           batch_size=B, num_heads=N_HEADS, _trace=False, _trace_kwargs=None):
    in_maps, has_bias, has_mask = shard_inputs(
        query, key, value, mask, Wq, bq, Wk, bk, Wv, bv, batch_size, num_heads)
    if has_bias or has_mask:
        # not exercised by this problem's inputs (zeros); keep a correct path
        return _reference_fallback(query, key, value, mask, Wq, bq, Wk, bk,
                                   Wv, bv, batch_size, num_heads)
    nc = _get_nc()
    res = run_bass_kernel_spmd(nc, in_maps, list(range(8)), trace=_trace,
                               **(_trace_kwargs or {}))
    full = assemble(res.results)
    if _trace:
        return full, res
    return full


# revision 55
# speedup vs baseline: 1.1960x; 1.1960x over previous
"""Multi-head attention (B=4, Q=K=2048, N=12 heads, H=64) on 8 TRN2 NeuronCores.

Sharding: core c handles batch b = c // 2 and head-group g = c % 2 (6 local
heads, output columns [g*384:(g+1)*384]). Pure data-parallel, no collectives.

v9 design (evolved from the v3 baseline via NTFF trace analysis; measures
~255us with the same tracing methodology that measured v3 at 328us):
  - Zero-padded per-(head, sub) q/k tiles: every matmul (scores, PV,
    projections) is a plain 128-deep full-array op, so the PE never
    changes tiling mode.  The 64x128 row-tiled score pairs of v3 never
    actually overlapped in production (trace-verified) and the constant
    (64,128)<->(128,128) mode flips cost ~100ns of drain per switch.
  - Split-sub score emission: each head's score matmuls are WAR-gated only
    on that head's own previous exp read (one sub-step earlier), so they
    run inside the other head's exp window and the Act engine (exp) never
    stalls on PE.
  - Wide-ones PV: v tiles are [128 = 64 v-dims | 64 ones] per head, so
    PSUM rows 64-127 of the PV accumulator hold broadcast copies of the
    softmax denominator.  The finish is: fast DVE copies (release the PSUM
    bank), one partition-stacked [128,512] reciprocal per head-pair in
    [128,128] quarters (DVE reciprocal costs ~6.3 cyc per FREE element,
    so partition-stacking both heads halves it), a DVE multiply, and a DMA
    to a TRANSPOSED DRAM output [384, 2048] (host assembles with .T).
    No PE transposes at all.  Finish work is drip-fed (<=2 DVE ops per
    stream entry) so it never delays the projection casts that gate PE.
  - 8 chunks of 2 k-tiles ([128,1024] score tiles = 2 PSUM banks) leave
    4 banks for DOUBLE-buffered projection and PV accumulators -- worth
    ~30us: with single buffers the Tile scheduler serializes the next
    (head, qb) PV/projection group behind the previous group's PSUM reads.
  - Inputs arrive host-packed chunk-major bf16 (6KB contiguous per
    partition per DMA) with weights dt-major; the prologue projects only
    what the first score chunk needs, and the remaining q/k projection
    fillers are spread as late as their score deadlines allow -- packing
    them early crowded the first third of the stream and stalled Act.
"""

import sys
from contextlib import ExitStack

sys.path.insert(0, "/opt/trn_rl_repo")

import numpy as np
import ml_dtypes

import concourse.bass as bass
import concourse.tile as tile
from concourse import bacc, mybir
from concourse.bass_utils import run_bass_kernel_spmd

F32 = mybir.dt.float32
BF16 = mybir.dt.bfloat16
EXPF = mybir.ActivationFunctionType.Exp
MUL = mybir.AluOpType.mult
DIV = mybir.AluOpType.divide

B, SEQ, N_HEADS, H = 4, 2048, 12, 64
D = N_HEADS * H            # 768
NH = 6                     # heads per core
NM = NH // 2               # head pairs (m-tiles)
DG = NH * H                # 384 output cols per core
P = 128
DT = D // P                # 6 d-tiles
QB = SEQ // 512            # 4 q blocks of 512
CHUNKS = (2, 2, 2, 2, 2, 2, 2, 2)   # k-tiles per chunk (exp width 1024)
NCH = len(CHUNKS)
RT = SEQ // P              # 16 k row tiles
E_LAG = 10                 # PV trails exp by this many stream entries
E_LAG_LATE = 3
LAG_SWITCH = 64
SCALE = 0.125              # 1/sqrt(64)
USE_DIVIDE = False         # DVE divide rejected by BIR verifier; recip+mul

NPBF16 = ml_dtypes.bfloat16


def build_nc(reps: int = 1, diag: str = ""):
    nc = bacc.Bacc("TRN2", target_bir_lowering=False, debug=False, num_devices=8)

    # chunk-major packed inputs: [ch][p][dt*512] -> 6KB contiguous per
    # partition per chunk, one fat DMA per (tensor, chunk)
    xq_d = nc.dram_tensor("xqT", [QB, P, DT * 512], BF16,
                          kind="ExternalInput").ap()
    xk_d = nc.dram_tensor("xkT", [QB, P, DT * 512], BF16,
                          kind="ExternalInput").ap()
    xv_d = nc.dram_tensor("xvT", [QB, P, DT * 512], BF16,
                          kind="ExternalInput").ap()
    x_d = {"q": xq_d, "k": xk_d, "v": xv_d}
    wq_d = nc.dram_tensor("wq", [NM, P, DT * P], BF16,
                          kind="ExternalInput").ap()
    wk_d = nc.dram_tensor("wk", [NM, P, DT * P], BF16,
                          kind="ExternalInput").ap()
    wv_d = nc.dram_tensor("wv", [P, DT * DG], BF16, kind="ExternalInput").ap()
    out_d = nc.dram_tensor("out", [DG, SEQ], F32, kind="ExternalOutput").ap()

    with tile.TileContext(nc) as tc:
     for _rep in range(reps):
      with ExitStack() as stack:
        singles = stack.enter_context(tc.tile_pool(name="singles", bufs=1))
        w_sb = {}
        for t in ("q", "k", "v"):
            w_sb[t] = singles.tile([P, DT, DG], BF16, tag=f"w{t}", name=f"w{t}")

        xTp = stack.enter_context(tc.tile_pool(name="xT", bufs=1))
        # per (tensor, chunk): [128, dt, 512] bf16
        xch = {(t, ch): xTp.tile([P, DT, 512], BF16, tag=f"{t}C{ch}",
                                 name=f"{t}C{ch}")
               for t in ("k", "q", "v") for ch in range(QB)}

        # projected q/k per (m-tile, sub-head): [128, seq] bf16 with the
        # OTHER head's 64 partitions zeroed.  Score matmuls are then plain
        # 128-deep full-array ops (zero rows contribute nothing), so the PE
        # never changes tiling mode -- no drain penalties, no tile_position.
        qkp = {(t, m, s): singles.tile([P, SEQ], BF16, tag=f"{t}m{m}s{s}",
                                       name=f"{t}m{m}s{s}")
               for t in ("q", "k") for m in range(NM) for s in (0, 1)}

        # v with wide-ones: [:, h, 0:64] = projected v, [:, h, 64:128] = 1.0
        vpool = stack.enter_context(tc.tile_pool(name="v", bufs=1))
        v_sb = [vpool.tile([P, NH, P], BF16, tag=f"v{kt}", name=f"v{kt}")
                for kt in range(RT)]
        for kt in range(RT):
            nc.gpsimd.memset(v_sb[kt][:, :, H:P], 1.0)

        # ---- input loads: k/w on the SP queue, q/v on the gpsimd (SWDGE)
        # queue so the two streams transfer in parallel ------------------
        def x_load_chunk(t, ch, eng):
            eng.dma_start(
                out=xch[t, ch].rearrange("p dt c -> p (dt c)"),
                in_=x_d[t][ch])

        def w_load(t, wd, m, eng):
            # m-major packed source: one contiguous 1.5KB/partition DMA
            eng.dma_start(out=w_sb[t][:, :, m * P:(m + 1) * P], in_=wd[m])
        w_load("k", wk_d, 0, nc.sync)
        x_load_chunk("k", 0, nc.sync)
        w_load("q", wq_d, 0, nc.sync)
        x_load_chunk("q", 0, nc.sync)
        for ch in range(1, 4):
            x_load_chunk("k", ch, nc.sync)
            x_load_chunk("q", ch, nc.sync)
        nc.sync.dma_start(out=w_sb["v"].rearrange("p dt c -> p (dt c)"),
                          in_=wv_d)
        for ch in range(4):
            x_load_chunk("v", ch, nc.sync)
        for m in range(1, NM):
            w_load("k", wk_d, m, nc.sync)
            w_load("q", wq_d, m, nc.sync)
        # zero the unused half of each per-sub q/k tile (after the DMA
        # triggers so the transfers start immediately; m0 first -- the
        # first scores need those)
        for m in range(NM):
            for t in ("k", "q"):
                nc.gpsimd.memset(qkp[t, m, 0][64:P, :], 0.0)
                nc.gpsimd.memset(qkp[t, m, 1][0:64, :], 0.0)

        # ---- main pools ----------------------------------------------------
        psProj = stack.enter_context(tc.tile_pool(name="psProj", bufs=1,
                                                  space="PSUM"))
        psS = stack.enter_context(tc.tile_pool(name="psS", bufs=1, space="PSUM"))
        psPV = stack.enter_context(tc.tile_pool(name="psPV", bufs=1,
                                                space="PSUM"))
        expp = stack.enter_context(tc.tile_pool(name="expp", bufs=E_LAG + 4))
        outp = stack.enter_context(tc.tile_pool(name="outp", bufs=3))
        smallp = stack.enter_context(tc.tile_pool(name="small", bufs=4))

        # ---- emission helpers ---------------------------------------------
        def proj_m(t, m, ch):
            """Project q/k m-tile chunk: q/k-range [ch*512,(ch+1)*512)."""
            pj = psProj.tile([P, 512], F32, tag="pj", name=f"pj{t}{m}{ch}", bufs=2)
            for dt in range(DT):
                nc.tensor.matmul(
                    pj, w_sb[t][:, dt, m * P:(m + 1) * P],
                    xch[t, ch][:, dt, :],
                    start=(dt == 0), stop=(dt == DT - 1))
            cs = slice(ch * 512, (ch + 1) * 512)
            nc.vector.tensor_copy(out=qkp[t, m, 0][0:64, cs], in_=pj[0:64, :])
            nc.vector.tensor_copy(out=qkp[t, m, 1][64:P, cs], in_=pj[64:P, :])

        def vproj_chunk(kt):
            pj = psProj.tile([P, 512], F32, tag="pj", name=f"pjv{kt}", bufs=2)
            ch, kk = divmod(kt, 4)
            for dt in range(DT):
                nc.tensor.matmul(pj[:, 0:DG],
                                 xch["v", ch][:, dt, kk * P:(kk + 1) * P],
                                 w_sb["v"][:, dt, :],
                                 start=(dt == 0), stop=(dt == DT - 1))
            nc.vector.tensor_copy(
                out=v_sb[kt][:, :, 0:H],
                in_=pj[:, 0:DG].rearrange("p (n h) -> p n h", h=H))

        # finish work is deferred and drip-fed (<=2 DVE ops per stream
        # entry) so its ~5us DVE burst never delays the projection casts
        # that gate PE work -- a burst at each (m,qb) boundary was stalling
        # the PE ~2.5us and re-throttling the HAM clock.
        dve_q = []

        def finish_pair(m, qb, pvA, pvB):
            """pv [128,512]: rows 0-63 context, 64-127 denominator copies.

            Fast DVE copies release the PSUM pv slots (keeps PE fed and the
            HAM clock warm).  The reciprocal costs ~6.3 DVE cycles per FREE
            element regardless of partitions, so both heads' denominators
            are stacked into one [128,512] tile and the reciprocal runs in
            [128,128] quarters to keep DVE queue blockages short.
            """
            hA, hB = 2 * m, 2 * m + 1
            last = (m, qb) == (NM - 1, QB - 1)
            den = smallp.tile([P, 512], F32, tag="den", name=f"d{m}{qb}",
                              bufs=2)
            if last:
                # end of kernel: multiply straight from PSUM, skip the
                # numerator staging copies (nothing left to unblock)
                num = {0: pvA, H: pvB}
                nc.vector.tensor_copy(out=den[0:H, :], in_=pvA[H:P, :])
                nc.vector.tensor_copy(out=den[H:P, :], in_=pvB[H:P, :])
            else:
                nt = outp.tile([P, 512], F32, tag="num", name=f"n{m}{qb}",
                               bufs=3)
                num = {0: nt[0:H, :], H: nt[H:P, :]}
                for pv, lo in ((pvA, 0), (pvB, H)):
                    nc.vector.tensor_copy(out=nt[lo:lo + H, :],
                                          in_=pv[0:H, :])
                    nc.vector.tensor_copy(out=den[lo:lo + H, :],
                                          in_=pv[H:P, :])
            rcp = smallp.tile([P, 512], F32, tag="rcp", name="rcp", bufs=2)

            def rq(qq):
                s = slice(qq * P, (qq + 1) * P)
                return lambda: nc.vector.reciprocal(rcp[:, s], den[:, s])

            def mq(h, lo, osb):
                def go():
                    nc.vector.tensor_tensor(out=osb[lo:lo + H, :],
                                            in0=(num[lo][0:H, :] if last
                                                 else num[lo]),
                                            in1=rcp[lo:lo + H, :], op=MUL)
                    nc.sync.dma_start(
                        out=out_d[h * H:(h + 1) * H,
                                  qb * 512:(qb + 1) * 512],
                        in_=osb[lo:lo + H, :])
                return go

            osb = outp.tile([P, 512], F32, tag="osb", name=f"o{m}{qb}",
                            bufs=3)
            ops = [rq(qq) for qq in range(4)] + [mq(hA, 0, osb),
                                                mq(hB, H, osb)]
            if last:
                for op in ops:     # last pair: straight-line, no deferral
                    op()
            else:
                dve_q.extend(ops)

        # ---- filler schedule (slot = stream entry index) -------------------
        # stream: (m, qb, c, sub) -> 12 entries per (m, qb); m0 spans
        # entries 0-47, m1 48-95, m2 96-143.
        def pj_item(t, m, ch):
            return lambda: proj_m(t, m, ch)

        def vp(k):
            return lambda: vproj_chunk(k)

        sched = {2: [pj_item("q", 0, 1)], 13: [pj_item("q", 0, 2)],
                 25: [pj_item("q", 0, 3)]}
        for k in range(RT):
            # vp k must land by stream entry 2*(k//3) + E_LAG (PV deadline;
            # sched items run before drain_pv within an entry)
            slot = 4 + k
            assert slot <= 2 * (k // 2) + E_LAG
            sched.setdefault(slot, []).append(vp(k))
        later = ([("k", 1, c) for c in range(4)] +
                 [("q", 1, c) for c in range(4)] +
                 [("k", 2, c) for c in range(4)] +
                 [("q", 2, c) for c in range(4)])
        slots_later = (26, 34, 42, 50, 56, 66, 76, 86,
                       94, 102, 110, 118, 126, 134, 142, 150)
        for i, (t, m, c) in enumerate(later):
            # deadlines: k-m1 by 64, q-m1 by 64+16c, k-m2 by 128, q-m2 by
            # 128+16c; each slot leaves >=8 entries of margin
            sched.setdefault(slots_later[i], []).append(pj_item(t, m, c))

        # ---- prologue: only what chunk-0 scores need; rest via sched ------
        proj_m("k", 0, 0)
        proj_m("q", 0, 0)
        # k m0 chunk ch feeds score chunks with k-tiles in [4ch, 4ch+4):
        # score chunk c uses kt 3c..3c+2 -> k-chunk 1 by entry 2, 2 by 4,
        # 3 by 8 (q m0 later chunks already in sched at 2/13/25)
        sched.setdefault(1, []).insert(0, pj_item("k", 0, 1))
        sched.setdefault(3, []).insert(0, pj_item("k", 0, 2))
        sched.setdefault(6, []).insert(0, pj_item("k", 0, 3))

        # ---- main loop -----------------------------------------------------
        stream = [(m, qb, c, sub) for m in range(NM) for qb in range(QB)
                  for c in range(NCH) for sub in (0, 1)]
        pv_tiles = {}
        e_tiles = {}

        KT0 = [sum(CHUNKS[:i]) for i in range(NCH)]

        def emit_pv(m, qb, c, sub):
            h = 2 * m + sub
            if (h, qb) not in pv_tiles:
                pv_tiles[h, qb] = psPV.tile([P, 512], F32, tag="pv",
                                            name=f"pv{h}{qb}", bufs=2)
            pv = pv_tiles[h, qb]
            e = e_tiles.pop((h, qb, c))
            for j in range(CHUNKS[c]):
                kt = KT0[c] + j
                nc.tensor.matmul(pv, v_sb[kt][:, h, :],
                                 e[:, j * 512:(j + 1) * 512],
                                 start=(kt == 0), stop=(kt == RT - 1))
            if c == NCH - 1 and sub == 1:
                finish_pair(m, qb, pv_tiles.pop((2 * m, qb)),
                            pv_tiles.pop((2 * m + 1, qb)))

        pv_next = [0]

        def drain_pv(idx):
            lag = E_LAG if idx < LAG_SWITCH else E_LAG_LATE
            limit = 3
            n = 0
            while pv_next[0] <= idx - lag and n < limit:
                emit_pv(*stream[pv_next[0]])
                pv_next[0] += 1
                n += 1

        # Each sub-head's score matmuls for chunk c are emitted at its own
        # sub step, WAR-gated only on THIS head's previous exp read (which
        # completed one sub-step earlier), so they run inside the other
        # head's exp window and the Act engine never stalls.  Zero-padded
        # qkp tiles make these plain 128-deep matmuls: no mode switches.
        for idx, (m, qb, c, sub) in enumerate(stream):
            csz = CHUNKS[c]
            h = 2 * m + sub
            sval = psS.tile([P, 1024], F32, tag=("sA" if sub == 0 else "sB"),
                            name=f"s{h}{qb}{c}")
            kTm, qTm = qkp["k", m, sub], qkp["q", m, sub]
            for j in range(csz):
                kt = KT0[c] + j
                nc.tensor.matmul(
                    sval[:, j * 512:(j + 1) * 512],
                    kTm[:, kt * P:(kt + 1) * P],
                    qTm[:, qb * 512:(qb + 1) * 512],
                    start=True, stop=True)
            e = expp.tile([P, 1024], BF16, tag="e", name="e")
            nc.scalar.activation(out=e[:, 0:csz * 512],
                                 in_=sval[:, 0:csz * 512],
                                 func=EXPF, scale=SCALE)
            e_tiles[h, qb, c] = e

            for item in sched.get(idx, ()):
                item()
            drain_pv(idx)
            for _ in range(6 if idx >= 168 else 2):
                if dve_q:
                    dve_q.pop(0)()

        while pv_next[0] < len(stream):
            emit_pv(*stream[pv_next[0]])
            pv_next[0] += 1
        while dve_q:
            dve_q.pop(0)()

    nc.compile()
    return nc


_NC_CACHE = {}


def _get_nc(has_bias=False, has_mask=False, reps=1, diag=""):
    assert not has_bias and not has_mask
    key = (reps, diag)
    if key not in _NC_CACHE:
        _NC_CACHE[key] = build_nc(reps, diag)
    return _NC_CACHE[key]


def _host_dtmajor(W):
    """[768, C] -> partition-major [128, 6*C]: row p holds dt-tile rows."""
    C = W.shape[1]
    return np.ascontiguousarray(
        W.reshape(DT, P, C).transpose(1, 0, 2).reshape(P, DT * C))


def _host_mmajor(W):
    """[768, 384] -> [NM, P, DT*128]: m-tile-major, contiguous per-m DMA."""
    return np.ascontiguousarray(
        W.reshape(DT, P, NM, P).transpose(2, 1, 0, 3).reshape(NM, P, DT * P))


def _host_chunkmajor(xb16_rows):
    """x [SEQ, D] bf16 -> [QB, P, DT*512]: chunk-major, 6KB/partition DMAs."""
    xT = xb16_rows.T  # [768, 2048]
    return np.ascontiguousarray(
        xT.reshape(DT, P, QB, 512).transpose(2, 1, 0, 3).reshape(
            QB, P, DT * 512))


def shard_inputs(query, key, value, mask, Wq, bq, Wk, bk, Wv, bv,
                 batch_size=B, num_heads=N_HEADS):
    query = np.asarray(query, dtype=np.float32)
    key = np.asarray(key, dtype=np.float32)
    value = np.asarray(value, dtype=np.float32)
    Wq = np.asarray(Wq, dtype=np.float32)
    Wk = np.asarray(Wk, dtype=np.float32)
    Wv = np.asarray(Wv, dtype=np.float32)
    assert query.shape == (B * SEQ, D) and key.shape == (B * SEQ, D)
    assert int(batch_size) == B and int(num_heads) == N_HEADS

    has_bias = bool(np.any(bq) or np.any(bk) or np.any(bv))
    has_mask = bool(np.any(mask))

    qb16 = query.astype(NPBF16)
    kb16 = key.astype(NPBF16)
    vb16 = value.astype(NPBF16)

    in_maps = []
    for c in range(8):
        b, g = divmod(c, 2)
        rows = slice(b * SEQ, (b + 1) * SEQ)
        cols = slice(g * DG, (g + 1) * DG)
        m = {
            "xqT": _host_chunkmajor(qb16[rows]),
            "xkT": _host_chunkmajor(kb16[rows]),
            "xvT": _host_chunkmajor(vb16[rows]),
            "wq": _host_mmajor(Wq[:, cols]).astype(NPBF16),
            "wk": _host_mmajor(Wk[:, cols]).astype(NPBF16),
            "wv": _host_dtmajor(Wv[:, cols]).astype(NPBF16),
        }
        in_maps.append(m)
    return in_maps, has_bias, has_mask


def make_in_maps(inputs):
    return shard_inputs(**{k: inputs[k] for k in
                           ("query", "key", "value", "mask", "Wq", "bq",
                            "Wk", "bk", "Wv", "bv", "batch_size", "num_heads")})[0]


def assemble(results):
    full = np.empty((B * SEQ, D), dtype=np.float32)
    for c in range(8):
        b, g = divmod(c, 2)
        full[b * SEQ:(b + 1) * SEQ, g * DG:(g + 1) * DG] = results[c]["out"].T
    return full


def _reference_fallback(query, key, value, mask, Wq, bq, Wk, bk, Wv, bv,
                        batch_size, num_heads):
    b, n = int(batch_size), int(num_heads)
    d = Wq.shape[1]
    h = d // n
    q_len = query.shape[0] // b
    k_len = key.shape[0] // b
    q = (query @ Wq + bq).reshape(b, q_len, n, h).transpose(0, 2, 1, 3)
    k = (key @ Wk + bk).reshape(b, k_len, n, h).transpose(0, 2, 1, 3)
    v = (value @ Wv + bv).reshape(b, k_len, n, h).transpose(0, 2, 1, 3)
    s = np.einsum('bnqh,bnkh->bnqk', q, k) / np.sqrt(h).astype(np.float32)
    s = s + mask
    s = s - s.max(-1, keepdims=True)
    w = np.exp(s)
    w /= w.sum(-1, keepdims=True)
    c = np.einsum('bnqk,bnkh->bqnh', w, v)
    return c.reshape(b * q_len, n * h).astype(np.float32)


def kernel(query, key, value, mask, Wq, bq, Wk, bk, Wv, bv,
           batch_size=B, num_heads=N_HEADS, _trace=False, _trace_kwargs=None):
    in_maps, has_bias, has_mask = shard_inputs(
        query, key, value, mask, Wq, bq, Wk, bk, Wv, bv, batch_size, num_heads)
    if has_bias or has_mask:
        # not exercised by this problem's inputs (zeros); keep a correct path
        return _reference_fallback(query, key, value, mask, Wq, bq, Wk, bk,
                                   Wv, bv, batch_size, num_heads)
    nc = _get_nc()
    res = run_bass_kernel_spmd(nc, in_maps, list(range(8)), trace=_trace,
                               **(_trace_kwargs or {}))
    full = assemble(res.results)
    if _trace:
        return full, res
    return full


# revision 56
# speedup vs baseline: 1.1995x; 1.0029x over previous
"""Multi-head attention (B=4, Q=K=2048, N=12 heads, H=64) on 8 TRN2 NeuronCores.

Sharding: core c handles batch b = c // 2 and head-group g = c % 2 (6 local
heads, output columns [g*384:(g+1)*384]). Pure data-parallel, no collectives.

v9 design (evolved from the v3 baseline via NTFF trace analysis; measures
~255us with the same tracing methodology that measured v3 at 328us):
  - Zero-padded per-(head, sub) q/k tiles: every matmul (scores, PV,
    projections) is a plain 128-deep full-array op, so the PE never
    changes tiling mode.  The 64x128 row-tiled score pairs of v3 never
    actually overlapped in production (trace-verified) and the constant
    (64,128)<->(128,128) mode flips cost ~100ns of drain per switch.
  - Split-sub score emission: each head's score matmuls are WAR-gated only
    on that head's own previous exp read (one sub-step earlier), so they
    run inside the other head's exp window and the Act engine (exp) never
    stalls on PE.
  - Wide-ones PV: v tiles are [128 = 64 v-dims | 64 ones] per head, so
    PSUM rows 64-127 of the PV accumulator hold broadcast copies of the
    softmax denominator.  The finish is: fast DVE copies (release the PSUM
    bank), one partition-stacked [128,512] reciprocal per head-pair in
    [128,128] quarters (DVE reciprocal costs ~6.3 cyc per FREE element,
    so partition-stacking both heads halves it), a DVE multiply, and a DMA
    to a TRANSPOSED DRAM output [384, 2048] (host assembles with .T).
    No PE transposes at all.  Finish work is drip-fed (<=2 DVE ops per
    stream entry) so it never delays the projection casts that gate PE.
  - 8 chunks of 2 k-tiles ([128,1024] score tiles = 2 PSUM banks) leave
    4 banks for DOUBLE-buffered projection and PV accumulators -- worth
    ~30us: with single buffers the Tile scheduler serializes the next
    (head, qb) PV/projection group behind the previous group's PSUM reads.
  - Inputs arrive host-packed chunk-major bf16 (6KB contiguous per
    partition per DMA) with weights dt-major; the prologue projects only
    what the first score chunk needs, and the remaining q/k projection
    fillers are spread as late as their score deadlines allow -- packing
    them early crowded the first third of the stream and stalled Act.
"""

import sys
from contextlib import ExitStack

sys.path.insert(0, "/opt/trn_rl_repo")

import numpy as np
import ml_dtypes

import concourse.bass as bass
import concourse.tile as tile
from concourse import bacc, mybir
from concourse.bass_utils import run_bass_kernel_spmd

F32 = mybir.dt.float32
BF16 = mybir.dt.bfloat16
EXPF = mybir.ActivationFunctionType.Exp
MUL = mybir.AluOpType.mult
DIV = mybir.AluOpType.divide

B, SEQ, N_HEADS, H = 4, 2048, 12, 64
D = N_HEADS * H            # 768
NH = 6                     # heads per core
NM = NH // 2               # head pairs (m-tiles)
DG = NH * H                # 384 output cols per core
P = 128
DT = D // P                # 6 d-tiles
QB = SEQ // 512            # 4 q blocks of 512
CHUNKS = (2, 2, 2, 2, 2, 2, 2, 2)   # k-tiles per chunk (exp width 1024)
NCH = len(CHUNKS)
RT = SEQ // P              # 16 k row tiles
E_LAG = 10                 # PV trails exp by this many stream entries
E_LAG_LATE = 3
LAG_SWITCH = 64
SCALE = 0.125              # 1/sqrt(64)
USE_DIVIDE = False         # DVE divide rejected by BIR verifier; recip+mul

NPBF16 = ml_dtypes.bfloat16


def build_nc(reps: int = 1, diag: str = ""):
    nc = bacc.Bacc("TRN2", target_bir_lowering=False, debug=False, num_devices=8)

    # chunk-major packed inputs: [ch][p][dt*512] -> 6KB contiguous per
    # partition per chunk, one fat DMA per (tensor, chunk)
    xq_d = nc.dram_tensor("xqT", [QB, P, DT * 512], BF16,
                          kind="ExternalInput").ap()
    xk_d = nc.dram_tensor("xkT", [QB, P, DT * 512], BF16,
                          kind="ExternalInput").ap()
    xv_d = nc.dram_tensor("xvT", [QB, P, DT * 512], BF16,
                          kind="ExternalInput").ap()
    x_d = {"q": xq_d, "k": xk_d, "v": xv_d}
    wq_d = nc.dram_tensor("wq", [NM, P, DT * P], BF16,
                          kind="ExternalInput").ap()
    wk_d = nc.dram_tensor("wk", [NM, P, DT * P], BF16,
                          kind="ExternalInput").ap()
    wv_d = nc.dram_tensor("wv", [P, DT * DG], BF16, kind="ExternalInput").ap()
    out_d = nc.dram_tensor("out", [DG, SEQ], F32, kind="ExternalOutput").ap()

    with tile.TileContext(nc) as tc:
     for _rep in range(reps):
      with ExitStack() as stack:
        singles = stack.enter_context(tc.tile_pool(name="singles", bufs=1))
        w_sb = {}
        for t in ("q", "k", "v"):
            w_sb[t] = singles.tile([P, DT, DG], BF16, tag=f"w{t}", name=f"w{t}")

        xTp = stack.enter_context(tc.tile_pool(name="xT", bufs=1))
        # per (tensor, chunk): [128, dt, 512] bf16
        xch = {(t, ch): xTp.tile([P, DT, 512], BF16, tag=f"{t}C{ch}",
                                 name=f"{t}C{ch}")
               for t in ("k", "q", "v") for ch in range(QB)}

        # projected q/k per (m-tile, sub-head): [128, seq] bf16 with the
        # OTHER head's 64 partitions zeroed.  Score matmuls are then plain
        # 128-deep full-array ops (zero rows contribute nothing), so the PE
        # never changes tiling mode -- no drain penalties, no tile_position.
        qkp = {(t, m, s): singles.tile([P, SEQ], BF16, tag=f"{t}m{m}s{s}",
                                       name=f"{t}m{m}s{s}")
               for t in ("q", "k") for m in range(NM) for s in (0, 1)}

        # v with wide-ones: [:, h, 0:64] = projected v, [:, h, 64:128] = 1.0
        vpool = stack.enter_context(tc.tile_pool(name="v", bufs=1))
        v_sb = [vpool.tile([P, NH, P], BF16, tag=f"v{kt}", name=f"v{kt}")
                for kt in range(RT)]
        for kt in range(RT):
            nc.gpsimd.memset(v_sb[kt][:, :, H:P], 1.0)

        # ---- input loads: k/w on the SP queue, q/v on the gpsimd (SWDGE)
        # queue so the two streams transfer in parallel ------------------
        def x_load_chunk(t, ch, eng):
            eng.dma_start(
                out=xch[t, ch].rearrange("p dt c -> p (dt c)"),
                in_=x_d[t][ch])

        def w_load(t, wd, m, eng):
            # m-major packed source: one contiguous 1.5KB/partition DMA
            eng.dma_start(out=w_sb[t][:, :, m * P:(m + 1) * P], in_=wd[m])
        w_load("k", wk_d, 0, nc.sync)
        x_load_chunk("k", 0, nc.sync)
        w_load("q", wq_d, 0, nc.sync)
        x_load_chunk("q", 0, nc.sync)
        for ch in range(1, 4):
            x_load_chunk("k", ch, nc.sync)
            x_load_chunk("q", ch, nc.sync)
        nc.sync.dma_start(out=w_sb["v"].rearrange("p dt c -> p (dt c)"),
                          in_=wv_d)
        for ch in range(4):
            x_load_chunk("v", ch, nc.sync)
        for m in range(1, NM):
            w_load("k", wk_d, m, nc.sync)
            w_load("q", wq_d, m, nc.sync)
        # zero the unused half of each per-sub q/k tile (after the DMA
        # triggers so the transfers start immediately; m0 first -- the
        # first scores need those)
        for m in range(NM):
            for t in ("k", "q"):
                nc.gpsimd.memset(qkp[t, m, 0][64:P, :], 0.0)
                nc.gpsimd.memset(qkp[t, m, 1][0:64, :], 0.0)

        # ---- main pools ----------------------------------------------------
        psProj = stack.enter_context(tc.tile_pool(name="psProj", bufs=1,
                                                  space="PSUM"))
        psS = stack.enter_context(tc.tile_pool(name="psS", bufs=1, space="PSUM"))
        psPV = stack.enter_context(tc.tile_pool(name="psPV", bufs=1,
                                                space="PSUM"))
        expp = stack.enter_context(tc.tile_pool(name="expp", bufs=E_LAG + 4))
        outp = stack.enter_context(tc.tile_pool(name="outp", bufs=3))
        smallp = stack.enter_context(tc.tile_pool(name="small", bufs=4))

        # ---- emission helpers ---------------------------------------------
        def proj_m(t, m, ch):
            """Project q/k m-tile chunk: q/k-range [ch*512,(ch+1)*512)."""
            pj = psProj.tile([P, 512], F32, tag="pj", name=f"pj{t}{m}{ch}", bufs=2)
            for dt in range(DT):
                nc.tensor.matmul(
                    pj, w_sb[t][:, dt, m * P:(m + 1) * P],
                    xch[t, ch][:, dt, :],
                    start=(dt == 0), stop=(dt == DT - 1))
            cs = slice(ch * 512, (ch + 1) * 512)
            nc.vector.tensor_copy(out=qkp[t, m, 0][0:64, cs], in_=pj[0:64, :])
            nc.vector.tensor_copy(out=qkp[t, m, 1][64:P, cs], in_=pj[64:P, :])

        def vproj_chunk(kt):
            pj = psProj.tile([P, 512], F32, tag="pj", name=f"pjv{kt}", bufs=2)
            ch, kk = divmod(kt, 4)
            for dt in range(DT):
                nc.tensor.matmul(pj[:, 0:DG],
                                 xch["v", ch][:, dt, kk * P:(kk + 1) * P],
                                 w_sb["v"][:, dt, :],
                                 start=(dt == 0), stop=(dt == DT - 1))
            nc.vector.tensor_copy(
                out=v_sb[kt][:, :, 0:H],
                in_=pj[:, 0:DG].rearrange("p (n h) -> p n h", h=H))

        # finish work is deferred and drip-fed (<=2 DVE ops per stream
        # entry) so its ~5us DVE burst never delays the projection casts
        # that gate PE work -- a burst at each (m,qb) boundary was stalling
        # the PE ~2.5us and re-throttling the HAM clock.
        dve_q = []

        def finish_pair(m, qb, pvA, pvB):
            """pv [128,512]: rows 0-63 context, 64-127 denominator copies.

            Fast DVE copies release the PSUM pv slots (keeps PE fed and the
            HAM clock warm).  The reciprocal costs ~6.3 DVE cycles per FREE
            element regardless of partitions, so both heads' denominators
            are stacked into one [128,512] tile and the reciprocal runs in
            [128,128] quarters to keep DVE queue blockages short.
            """
            hA, hB = 2 * m, 2 * m + 1
            last = (m, qb) == (NM - 1, QB - 1)
            den = smallp.tile([P, 512], F32, tag="den", name=f"d{m}{qb}",
                              bufs=2)
            if last:
                # end of kernel: multiply straight from PSUM, skip the
                # numerator staging copies (nothing left to unblock)
                num = {0: pvA, H: pvB}
                nc.vector.tensor_copy(out=den[0:H, :], in_=pvA[H:P, :])
                nc.vector.tensor_copy(out=den[H:P, :], in_=pvB[H:P, :])
            else:
                nt = outp.tile([P, 512], F32, tag="num", name=f"n{m}{qb}",
                               bufs=3)
                num = {0: nt[0:H, :], H: nt[H:P, :]}
                for pv, lo in ((pvA, 0), (pvB, H)):
                    nc.vector.tensor_copy(out=nt[lo:lo + H, :],
                                          in_=pv[0:H, :])
                    nc.vector.tensor_copy(out=den[lo:lo + H, :],
                                          in_=pv[H:P, :])
            rcp = smallp.tile([P, 512], F32, tag="rcp", name="rcp", bufs=2)

            def rq(qq):
                s = slice(qq * P, (qq + 1) * P)
                return lambda: nc.vector.reciprocal(rcp[:, s], den[:, s])

            def mq(h, lo, osb):
                def go():
                    nc.vector.tensor_tensor(out=osb[lo:lo + H, :],
                                            in0=(num[lo][0:H, :] if last
                                                 else num[lo]),
                                            in1=rcp[lo:lo + H, :], op=MUL)
                    nc.sync.dma_start(
                        out=out_d[h * H:(h + 1) * H,
                                  qb * 512:(qb + 1) * 512],
                        in_=osb[lo:lo + H, :])
                return go

            osb = outp.tile([P, 512], F32, tag="osb", name=f"o{m}{qb}",
                            bufs=3)
            ops = [rq(qq) for qq in range(4)] + [mq(hA, 0, osb),
                                                mq(hB, H, osb)]
            if last:
                for op in ops:     # last pair: straight-line, no deferral
                    op()
            else:
                dve_q.extend(ops)

        # ---- filler schedule (slot = stream entry index) -------------------
        # stream: (m, qb, c, sub) -> 12 entries per (m, qb); m0 spans
        # entries 0-47, m1 48-95, m2 96-143.
        def pj_item(t, m, ch):
            return lambda: proj_m(t, m, ch)

        def vp(k):
            return lambda: vproj_chunk(k)

        sched = {2: [pj_item("q", 0, 1)], 13: [pj_item("q", 0, 2)],
                 25: [pj_item("q", 0, 3)]}
        for k in range(RT):
            # vp k must land by stream entry 2*(k//3) + E_LAG (PV deadline;
            # sched items run before drain_pv within an entry)
            slot = 4 + k
            assert slot <= 2 * (k // 2) + E_LAG
            sched.setdefault(slot, []).append(vp(k))
        later = ([("k", 1, c) for c in range(4)] +
                 [("q", 1, c) for c in range(4)] +
                 [("k", 2, c) for c in range(4)] +
                 [("q", 2, c) for c in range(4)])
        slots_later = (26, 34, 42, 50, 56, 66, 76, 86,
                       94, 102, 110, 118, 126, 134, 142, 150)
        for i, (t, m, c) in enumerate(later):
            # deadlines: k-m1 by 64, q-m1 by 64+16c, k-m2 by 128, q-m2 by
            # 128+16c; each slot leaves >=8 entries of margin
            sched.setdefault(slots_later[i], []).append(pj_item(t, m, c))

        # ---- prefetch the exp activation table during the DMA ramp: the
        # ACT_TABLE_LOAD (~1.3-2.7us) otherwise lands on the first real exp
        warm_in = smallp.tile([1, 8], F32, tag="wrmA", name="warm_in", bufs=1)
        warm_out = smallp.tile([1, 8], F32, tag="wrmB", name="warm_out",
                               bufs=1)
        nc.vector.memset(warm_in, 0.0)
        nc.scalar.activation(out=warm_out, in_=warm_in, func=EXPF, scale=1.0)

        # ---- prologue: only what chunk-0 scores need; rest via sched ------
        proj_m("k", 0, 0)
        proj_m("q", 0, 0)
        # k m0 chunk ch feeds score chunks with k-tiles in [4ch, 4ch+4):
        # score chunk c uses kt 3c..3c+2 -> k-chunk 1 by entry 2, 2 by 4,
        # 3 by 8 (q m0 later chunks already in sched at 2/13/25)
        sched.setdefault(1, []).insert(0, pj_item("k", 0, 1))
        sched.setdefault(3, []).insert(0, pj_item("k", 0, 2))
        sched.setdefault(6, []).insert(0, pj_item("k", 0, 3))

        # ---- main loop -----------------------------------------------------
        stream = [(m, qb, c, sub) for m in range(NM) for qb in range(QB)
                  for c in range(NCH) for sub in (0, 1)]
        pv_tiles = {}
        e_tiles = {}

        KT0 = [sum(CHUNKS[:i]) for i in range(NCH)]

        def emit_pv(m, qb, c, sub):
            h = 2 * m + sub
            if (h, qb) not in pv_tiles:
                pv_tiles[h, qb] = psPV.tile([P, 512], F32, tag="pv",
                                            name=f"pv{h}{qb}", bufs=2)
            pv = pv_tiles[h, qb]
            e = e_tiles.pop((h, qb, c))
            for j in range(CHUNKS[c]):
                kt = KT0[c] + j
                nc.tensor.matmul(pv, v_sb[kt][:, h, :],
                                 e[:, j * 512:(j + 1) * 512],
                                 start=(kt == 0), stop=(kt == RT - 1))
            if c == NCH - 1 and sub == 1:
                finish_pair(m, qb, pv_tiles.pop((2 * m, qb)),
                            pv_tiles.pop((2 * m + 1, qb)))

        pv_next = [0]

        def drain_pv(idx):
            lag = E_LAG if idx < LAG_SWITCH else E_LAG_LATE
            limit = 3
            n = 0
            while pv_next[0] <= idx - lag and n < limit:
                emit_pv(*stream[pv_next[0]])
                pv_next[0] += 1
                n += 1

        # Each sub-head's score matmuls for chunk c are emitted at its own
        # sub step, WAR-gated only on THIS head's previous exp read (which
        # completed one sub-step earlier), so they run inside the other
        # head's exp window and the Act engine never stalls.  Zero-padded
        # qkp tiles make these plain 128-deep matmuls: no mode switches.
        for idx, (m, qb, c, sub) in enumerate(stream):
            csz = CHUNKS[c]
            h = 2 * m + sub
            sval = psS.tile([P, 1024], F32, tag=("sA" if sub == 0 else "sB"),
                            name=f"s{h}{qb}{c}")
            kTm, qTm = qkp["k", m, sub], qkp["q", m, sub]
            for j in range(csz):
                kt = KT0[c] + j
                nc.tensor.matmul(
                    sval[:, j * 512:(j + 1) * 512],
                    kTm[:, kt * P:(kt + 1) * P],
                    qTm[:, qb * 512:(qb + 1) * 512],
                    start=True, stop=True)
            e = expp.tile([P, 1024], BF16, tag="e", name="e")
            nc.scalar.activation(out=e[:, 0:csz * 512],
                                 in_=sval[:, 0:csz * 512],
                                 func=EXPF, scale=SCALE)
            e_tiles[h, qb, c] = e

            for item in sched.get(idx, ()):
                item()
            drain_pv(idx)
            for _ in range(6 if idx >= 168 else 2):
                if dve_q:
                    dve_q.pop(0)()

        while pv_next[0] < len(stream):
            emit_pv(*stream[pv_next[0]])
            pv_next[0] += 1
        while dve_q:
            dve_q.pop(0)()

    nc.compile()
    return nc


_NC_CACHE = {}


def _get_nc(has_bias=False, has_mask=False, reps=1, diag=""):
    assert not has_bias and not has_mask
    key = (reps, diag)
    if key not in _NC_CACHE:
        _NC_CACHE[key] = build_nc(reps, diag)
    return _NC_CACHE[key]


def _host_dtmajor(W):
    """[768, C] -> partition-major [128, 6*C]: row p holds dt-tile rows."""
    C = W.shape[1]
    return np.ascontiguousarray(
        W.reshape(DT, P, C).transpose(1, 0, 2).reshape(P, DT * C))


def _host_mmajor(W):
    """[768, 384] -> [NM, P, DT*128]: m-tile-major, contiguous per-m DMA."""
    return np.ascontiguousarray(
        W.reshape(DT, P, NM, P).transpose(2, 1, 0, 3).reshape(NM, P, DT * P))


def _host_chunkmajor(xb16_rows):
    """x [SEQ, D] bf16 -> [QB, P, DT*512]: chunk-major, 6KB/partition DMAs."""
    xT = xb16_rows.T  # [768, 2048]
    return np.ascontiguousarray(
        xT.reshape(DT, P, QB, 512).transpose(2, 1, 0, 3).reshape(
            QB, P, DT * 512))


def shard_inputs(query, key, value, mask, Wq, bq, Wk, bk, Wv, bv,
                 batch_size=B, num_heads=N_HEADS):
    query = np.asarray(query, dtype=np.float32)
    key = np.asarray(key, dtype=np.float32)
    value = np.asarray(value, dtype=np.float32)
    Wq = np.asarray(Wq, dtype=np.float32)
    Wk = np.asarray(Wk, dtype=np.float32)
    Wv = np.asarray(Wv, dtype=np.float32)
    assert query.shape == (B * SEQ, D) and key.shape == (B * SEQ, D)
    assert int(batch_size) == B and int(num_heads) == N_HEADS

    has_bias = bool(np.any(bq) or np.any(bk) or np.any(bv))
    has_mask = bool(np.any(mask))

    qb16 = query.astype(NPBF16)
    kb16 = key.astype(NPBF16)
    vb16 = value.astype(NPBF16)

    in_maps = []
    for c in range(8):
        b, g = divmod(c, 2)
        rows = slice(b * SEQ, (b + 1) * SEQ)
        cols = slice(g * DG, (g + 1) * DG)
        m = {
            "xqT": _host_chunkmajor(qb16[rows]),
            "xkT": _host_chunkmajor(kb16[rows]),
            "xvT": _host_chunkmajor(vb16[rows]),
            "wq": _host_mmajor(Wq[:, cols]).astype(NPBF16),
            "wk": _host_mmajor(Wk[:, cols]).astype(NPBF16),
            "wv": _host_dtmajor(Wv[:, cols]).astype(NPBF16),
        }
        in_maps.append(m)
    return in_maps, has_bias, has_mask


def make_in_maps(inputs):
    return shard_inputs(**{k: inputs[k] for k in
                           ("query", "key", "value", "mask", "Wq", "bq",
                            "Wk", "bk", "Wv", "bv", "batch_size", "num_heads")})[0]


def assemble(results):
    full = np.empty((B * SEQ, D), dtype=np.float32)
    for c in range(8):
        b, g = divmod(c, 2)
        full[b * SEQ:(b + 1) * SEQ, g * DG:(g + 1) * DG] = results[c]["out"].T
    return full


def _reference_fallback(query, key, value, mask, Wq, bq, Wk, bk, Wv, bv,
                        batch_size, num_heads):
    b, n = int(batch_size), int(num_heads)
    d = Wq.shape[1]
    h = d // n
    q_len = query.shape[0] // b
    k_len = key.shape[0] // b
    q = (query @ Wq + bq).reshape(b, q_len, n, h).transpose(0, 2, 1, 3)
    k = (key @ Wk + bk).reshape(b, k_len, n, h).transpose(0, 2, 1, 3)
    v = (value @ Wv + bv).reshape(b, k_len, n, h).transpose(0, 2, 1, 3)
    s = np.einsum('bnqh,bnkh->bnqk', q, k) / np.sqrt(h).astype(np.float32)
    s = s + mask
    s = s - s.max(-1, keepdims=True)
    w = np.exp(s)
    w /= w.sum(-1, keepdims=True)
    c = np.einsum('bnqk,bnkh->bqnh', w, v)
    return c.reshape(b * q_len, n * h).astype(np.float32)


def kernel(query, key, value, mask, Wq, bq, Wk, bk, Wv, bv,
           batch_size=B, num_heads=N_HEADS, _trace=False, _trace_kwargs=None):
    in_maps, has_bias, has_mask = shard_inputs(
        query, key, value, mask, Wq, bq, Wk, bk, Wv, bv, batch_size, num_heads)
    if has_bias or has_mask:
        # not exercised by this problem's inputs (zeros); keep a correct path
        return _reference_fallback(query, key, value, mask, Wq, bq, Wk, bk,
                                   Wv, bv, batch_size, num_heads)
    nc = _get_nc()
    res = run_bass_kernel_spmd(nc, in_maps, list(range(8)), trace=_trace,
                               **(_trace_kwargs or {}))
    full = assemble(res.results)
    if _trace:
        return full, res
    return full
